# revision 3
# baseline (speedup 1.0000x reference)
"""Trainium2 8-core multi-head attention kernel (nn_Attention_670014898316).

B=1, S=4096, E=768, H=12 heads of D=64.

Sharding: sequence-parallel over queries (512 rows/core). Each core projects
K/V for its own 512-row shard, AllGathers the bf16 shards (K first, then V, so
score matmuls can start while V is still in flight), then computes its
queries' full attention over all 4096 keys for all 12 heads, plus the output
projection. The host concatenates the per-core output rows — no all-reduce.

All device matmuls run in bf16 with fp32 PSUM accumulation. Everything is kept
in transposed [E, S] orientation so no on-chip transposes are needed. Softmax
skips max-subtraction (scores are ~N(0,1); exp cannot overflow fp32) and the
softmax denominator rides the ctx matmul as a 65th all-ones row of V.
"""

import sys

if "/opt/trn_rl_repo" not in sys.path:
    sys.path.insert(0, "/opt/trn_rl_repo")

import numpy as np
import ml_dtypes

import concourse.bass as bass
import concourse.mybir as mybir
import concourse.tile as tile
from concourse import bacc, bass_utils

BF16 = mybir.dt.bfloat16
F32 = mybir.dt.float32

B, S, E, H, D = 1, 4096, 768, 12, 64
N_CORES = 8
SC = S // N_CORES          # 512 query rows per core
C = E // 128               # 6 partition chunks of the embedding dim
NPAIR = H // 2             # 6 head pairs
KSZ = E * SC               # elements in one K_T (or V) shard
GROUP = 3                  # score k-chunks per ACT exp instruction


def _build():
    nc = bacc.Bacc("TRN2", target_bir_lowering=False, debug=False,
                   num_devices=N_CORES)

    xqT = nc.dram_tensor("xqT", [E, SC], BF16, kind="ExternalInput")
    xkT = nc.dram_tensor("xkT", [E, SC], BF16, kind="ExternalInput")
    xvT = nc.dram_tensor("xvT", [E, SC], BF16, kind="ExternalInput")
    wqT = nc.dram_tensor("wqT", [E, E], BF16, kind="ExternalInput")
    wkT = nc.dram_tensor("wkT", [E, E], BF16, kind="ExternalInput")
    wvT = nc.dram_tensor("wvT", [E, E], BF16, kind="ExternalInput")
    woT = nc.dram_tensor("woT", [E, E], BF16, kind="ExternalInput")
    outT = nc.dram_tensor("out", [E, SC], F32, kind="ExternalOutput")

    cck_in = nc.dram_tensor("cck_in", [KSZ], BF16)
    ccv_in = nc.dram_tensor("ccv_in", [KSZ], BF16)
    cck_out = nc.dram_tensor("cck_out", [KSZ * N_CORES], BF16,
                             addr_space="Shared")
    ccv_out = nc.dram_tensor("ccv_out", [KSZ * N_CORES], BF16,
                             addr_space="Shared")

    def load_chunked(pool, dram, ncols, name):
        """Load [E, ncols] dram tensor as [128, C, ncols], one DMA per chunk."""
        t = pool.tile([128, C, ncols], BF16, name=name)
        for ci in range(C):
            nc.sync.dma_start(
                t[:, ci, :],
                bass.AP(tensor=dram, offset=128 * ci * ncols,
                        ap=[[ncols, 128], [1, ncols]]))
        return t

    with tile.TileContext(nc) as tc:
        with (
            tc.tile_pool(name="persist", bufs=1) as persist,
            tc.tile_pool(name="rs_dram", bufs=2, space="DRAM") as rs_dram,
        ):
            qT = persist.tile([128, C, SC], BF16)      # Q^T, full per core
            ctxT = persist.tile([128, C, SC], BF16)    # normalized context^T

            # ---------------- Phase A: projections + AllGathers ----------------
            with (
                tc.tile_pool(name="pa_sb", bufs=1) as pa,
                tc.tile_pool(name="pa_ps", bufs=2, space="PSUM") as pps,
            ):
                xk_t = load_chunked(pa, xkT, SC, "xk_t")
                wk_t = load_chunked(pa, wkT, E, "wk_t")
                xv_t = load_chunked(pa, xvT, SC, "xv_t")
                wv_t = load_chunked(pa, wvT, E, "wv_t")
                xq_t = load_chunked(pa, xqT, SC, "xq_t")
                wq_t = load_chunked(pa, wqT, E, "wq_t")

                kT_sh = pa.tile([128, C, SC], BF16)    # K^T shard [768, 512]
                v_sh = pa.tile([128, SC // 128, E], BF16)  # V shard [512, 768]

                # K^T = wkT.T @ xk (chunked), AG_K; then V, AG_V; then Q.
                for mo in range(C):
                    pt = pps.tile([128, E], F32, tag="paps")
                    for ki in range(C):
                        nc.tensor.matmul(pt[:, 0:SC],
                                         wk_t[:, ki, 128 * mo:128 * mo + 128],
                                         xk_t[:, ki, :],
                                         start=(ki == 0), stop=(ki == C - 1))
                    nc.vector.tensor_copy(kT_sh[:, mo, :], pt[:, 0:SC])
                    nc.sync.dma_start(
                        bass.AP(tensor=cck_in, offset=128 * mo * SC,
                                ap=[[SC, 128], [1, SC]]),
                        kT_sh[:, mo, :])
                nc.gpsimd.collective_compute(
                    "AllGather", mybir.AluOpType.bypass,
                    replica_groups=[list(range(N_CORES))],
                    ins=[cck_in.ap()], outs=[cck_out.ap()],
                )

                for si in range(SC // 128):
                    pt = pps.tile([128, E], F32, tag="paps")
                    for n0, n1 in ((0, 512), (512, 768)):
                        for ki in range(C):
                            nc.tensor.matmul(pt[:, n0:n1],
                                             xv_t[:, ki, 128 * si:128 * si + 128],
                                             wv_t[:, ki, n0:n1],
                                             start=(ki == 0), stop=(ki == C - 1))
                    nc.vector.tensor_copy(v_sh[:, si, :], pt[:])
                    nc.sync.dma_start(
                        bass.AP(tensor=ccv_in, offset=128 * si * E,
                                ap=[[E, 128], [1, E]]),
                        v_sh[:, si, :])
                nc.gpsimd.collective_compute(
                    "AllGather", mybir.AluOpType.bypass,
                    replica_groups=[list(range(N_CORES))],
                    ins=[ccv_in.ap()], outs=[ccv_out.ap()],
                )

                for mo in range(C):
                    pt = pps.tile([128, E], F32, tag="paps")
                    for ki in range(C):
                        nc.tensor.matmul(pt[:, 0:SC],
                                         wq_t[:, ki, 128 * mo:128 * mo + 128],
                                         xq_t[:, ki, :],
                                         start=(ki == 0), stop=(ki == C - 1))
                    nc.vector.tensor_copy(qT[:, mo, :], pt[:, 0:SC])

            # ---------------- Phase C: attention, one head-pair at a time ----
            nchunk = S // 128  # 32 key chunks
            groups = [list(range(g, min(g + GROUP, nchunk)))
                      for g in range(0, nchunk, GROUP)]

            with (
                tc.tile_pool(name="pc_kv", bufs=3) as kv,
                tc.tile_pool(name="pc_pt", bufs=6) as ptp,
                tc.tile_pool(name="pc_misc", bufs=4) as msc,
                tc.tile_pool(name="pc_ps_s", bufs=2, space="PSUM") as psS,
                tc.tile_pool(name="pc_ps_c", bufs=2, space="PSUM") as psC,
            ):
                for h2 in range(NPAIR):
                    # K^T rows for this pair: [128, 4096] (head A = rows 0:64)
                    kT_p = kv.tile([128, S], BF16, tag="kT")
                    for r in range(N_CORES):
                        nc.sync.dma_start(
                            kT_p[:, SC * r:SC * (r + 1)],
                            bass.AP(tensor=cck_out,
                                    offset=KSZ * r + 128 * h2 * SC,
                                    ap=[[SC, 128], [1, SC]]))
                    # V columns for this pair, ones-augmented: [128, 32, 130]
                    v_p = kv.tile([128, nchunk, 2 * (D + 1)], BF16, tag="v")
                    for r in range(N_CORES):
                        for hh in range(2):
                            nc.sync.dma_start(
                                v_p[:, 4 * r:4 * r + 4,
                                    (D + 1) * hh:(D + 1) * hh + D],
                                bass.AP(tensor=ccv_out,
                                        offset=(KSZ * r + D * (2 * h2 + hh)),
                                        ap=[[E, 128], [128 * E, 4], [1, D]]))
                    ones_view = v_p.rearrange("p c (h e) -> p c h e", h=2)
                    nc.vector.memset(ones_view[:, :, :, D:D + 1], 1.0)

                    ctx = [psC.tile([D + 1, SC], F32, tag="ctx", name=f"ctx{_hh}")
                           for _hh in range(2)]

                    for g in groups:
                        L = len(g)
                        pT = ptp.tile([128, 2 * GROUP, SC], BF16, tag="pT")
                        Sp = [psS.tile([128, GROUP, SC], F32, tag="S", name=f"S{_hh}")
                              for _hh in range(2)]
                        for hh in range(2):
                            p0, p1 = 64 * hh, 64 * hh + 64
                            for i, kc in enumerate(g):
                                nc.tensor.matmul(
                                    Sp[hh][:, i, :],
                                    kT_p[p0:p1, 128 * kc:128 * kc + 128],
                                    qT[p0:p1, h2, :],
                                    start=True, stop=True,
                                    tile_position=(64 * hh, 0))
                            nc.scalar.activation(
                                pT[:, GROUP * hh:GROUP * hh + L, :],
                                Sp[hh][:, 0:L, :],
                                mybir.ActivationFunctionType.Exp, scale=1.0 / 8.0)
                        for hh in range(2):
                            for i, kc in enumerate(g):
                                nc.tensor.matmul(
                                    ctx[hh],
                                    v_p[:, kc, (D + 1) * hh:(D + 1) * (hh + 1)],
                                    pT[:, GROUP * hh + i, :],
                                    start=(kc == 0), stop=(kc == nchunk - 1))

                    # normalize: ctx rows 0:64 are context^T, row 64 is rowsum
                    for hh in range(2):
                        rrow = msc.tile([1, SC], F32, tag="rrow")
                        nc.vector.reciprocal(rrow[:], ctx[hh][D:D + 1, :])
                        rs_b = rs_dram.tile([SC], F32)
                        nc.sync.dma_start(rs_b[:], rrow[:])
                        bcast = msc.tile([D, SC], F32, tag="bcast")
                        nc.sync.dma_start(
                            bcast[:],
                            bass.AP(tensor=rs_b.tensor, offset=rs_b.offset,
                                    ap=[[0, D], [1, SC]]))
                        if hh == 0:
                            nc.vector.tensor_mul(ctxT[0:D, h2, :],
                                                 ctx[hh][0:D, :], bcast[:])
                        else:
                            stg = msc.tile([D, SC], BF16, tag="stg")
                            nc.vector.tensor_mul(stg[:], ctx[hh][0:D, :], bcast[:])
                            nc.sync.dma_start(ctxT[D:128, h2, :], stg[:])

            # ---------------- Phase D: output projection ----------------
            with (
                tc.tile_pool(name="pd_sb", bufs=2) as pd,
                tc.tile_pool(name="pd_ps", bufs=2, space="PSUM") as pdps,
            ):
                wo_t = load_chunked(pd, woT, E, "wo_t")
                for mo in range(C):
                    pt = pdps.tile([128, SC], F32, tag="pdps")
                    for ki in range(C):
                        nc.tensor.matmul(pt[:],
                                         wo_t[:, ki, 128 * mo:128 * mo + 128],
                                         ctxT[:, ki, :],
                                         start=(ki == 0), stop=(ki == C - 1))
                    st = pd.tile([128, SC], F32, tag="pdst")
                    nc.vector.tensor_copy(st[:], pt[:])
                    nc.sync.dma_start(
                        bass.AP(tensor=outT, offset=128 * mo * SC,
                                ap=[[SC, 128], [1, SC]]),
                        st[:])

    nc.compile()
    return nc


_NC_CACHE = None


def _get_module():
    global _NC_CACHE
    if _NC_CACHE is None:
        _NC_CACHE = _build()
    return _NC_CACHE


def _bf16(a):
    return np.asarray(a, dtype=np.float32).astype(ml_dtypes.bfloat16)


def kernel(inputQueries, inputKeys, inputValues, Wq, Wk, Wv, Wo, _trace=False):
    nc = _get_module()

    wqT = np.ascontiguousarray(_bf16(np.asarray(Wq).T))
    wkT = np.ascontiguousarray(_bf16(np.asarray(Wk).T))
    wvT = np.ascontiguousarray(_bf16(np.asarray(Wv).T))
    woT = np.ascontiguousarray(_bf16(np.asarray(Wo).T))

    xq = np.asarray(inputQueries).reshape(S, E)
    xk = np.asarray(inputKeys).reshape(S, E)
    xv = np.asarray(inputValues).reshape(S, E)

    in_maps = []
    for c in range(N_CORES):
        rows = slice(SC * c, SC * (c + 1))
        in_maps.append({
            "xqT": np.ascontiguousarray(_bf16(xq[rows]).T),
            "xkT": np.ascontiguousarray(_bf16(xk[rows]).T),
            "xvT": np.ascontiguousarray(_bf16(xv[rows]).T),
            "wqT": wqT, "wkT": wkT, "wvT": wvT, "woT": woT,
        })

    res = bass_utils.run_bass_kernel_spmd(
        nc, in_maps, core_ids=list(range(N_CORES)), trace=_trace)

    out = np.empty((B, S, E), dtype=np.float32)
    for c in range(N_CORES):
        out[0, SC * c:SC * (c + 1), :] = res.results[c]["out"].T
    if _trace:
        return out, res
    return out
